# revision 9
# baseline (speedup 1.0000x reference)
"""Trainium2 Bass kernel for the NEUROPULS photonic-mesh transfer matrix.

The reference's crossing layers are discarded, so the full 512x512 transfer
matrix is block-diagonal with 256 independent 2x2 complex blocks:

    G_k = E_out(k) . Prod_{i=255..0} S_i(k) . E_in(k),
    S_i = B(2i+1) . diag(e^{i phi}) . B(2i),   B = [[t, i k], [i k, t]]

Layout: pairs live in PARTITIONS (pair k = 128*h + p, h packed in the free
dim), per-step data in the FREE dim.  Construction (amplitudes, trig, step
matrix entries) and the binary combine tree of 2x2 complex products then share
one partition assignment -- no relayout, no PE, no PSUM: everything is a
DVE/ACT/Pool elementwise op over affine views.

Steps are stored in bit-reversed lane order, so at every tree level the left
(odd) and right (even) factors are contiguous blocks and the last view dim
stays packed -- enabling the DVE 2x 16-bit mode for the fp16 combine tree.
The amplitude exponential 10^(-l/20) is a DVE cubic (|x| <= 0.058), so only
the Sin activation-table set is ever loaded (one 1283 ns load, hoisted under
the input-DMA latency).

Step-matrix entries (with primed trig c' = -cos, s' = -sin; S' = -S and the
sign cancels over the even number of steps):
    S00 = tt*ca - kk*cb, S01 = i*(tk*ca + kt*cb), ...  (tt = t1*t0 etc.)
computed as one outer product PR[cq, taut] = coef[cq] * trig[taut] plus four
paired add/sub ops over strided views.  Each core combines its 32 steps down
to 4 partials per pair; the host chains the 8x4 partials and scatters the
2x2 blocks into the zero matrix.
"""

import sys

sys.path.insert(0, "/opt/trn_rl_repo")

import numpy as np

N = 512
NPAIR = 256
NCORE = 8
CH = 32  # steps per core
TWO_PI = 2.0 * np.pi

_BR5 = np.array([int(f"{i:05b}"[::-1], 2) for i in range(32)])  # bit-reverse
_BR2 = np.array([0, 2, 1, 3])

# ---------------------------------------------------------------------------
# host-side shard prep / final combine
# ---------------------------------------------------------------------------


def _host_prep(core, losses, imbal, phases):
    """Per-core DRAM input [128, 384].

    Partition p, free u = 32*h + bitrev5(i) (h: pair-half, i: local step),
    pair k = 128*h + p, global step ig = 32*core + i.
    Free blocks of 64: [l0 | l1 | m0 | m1 | alpha | beta].
    """
    i = np.arange(CH) + CH * core  # (32,) global step
    ii = i[None, None, :]  # (1,1,32)
    kk = (np.arange(2)[None, :, None] * 128 + np.arange(128)[:, None, None])  # (128,2,1)

    def pack(block):  # (128, 2, 32) -> (128, 64), free u = bitrev5(i)*2 + h
        out = np.empty((128, 32, 2), np.float32)
        out[:, _BR5, :] = block.transpose(0, 2, 1)
        return out.reshape(128, 64)

    INP = np.empty((128, 384), np.float32)
    INP[:, 0:64] = pack(losses[2 * ii, kk])
    INP[:, 64:128] = pack(losses[2 * ii + 1, kk])
    INP[:, 128:192] = pack(imbal[2 * ii, kk])
    INP[:, 192:256] = pack(imbal[2 * ii + 1, kk])
    INP[:, 256:320] = pack(phases[ii, 2 * kk])
    INP[:, 320:384] = pack(phases[ii, 2 * kk + 1])
    return INP


def _host_finish(Os, phases_in, phases_out):
    """Chain the per-core 4-step partials and scatter into the full matrix.

    Os[c]: (128, 64) fp16, free idx = comp*8 + pos*2 + h with comp = 4r+2s+part
    and pos = bitrev2(jp); partial jp covers steps [8jp, 8jp+8) (later steps =
    applied on the left).
    """
    M = np.tile(np.eye(2, dtype=np.complex128), (NPAIR, 1, 1))
    for c in range(NCORE):
        v = Os[c].astype(np.float64).reshape(128, 2, 2, 2, 4, 2)  # p,r,s,part,pos,h
        v = v[:, :, :, :, _BR2, :]  # jp order
        G = v[:, :, :, 0] + 1j * v[:, :, :, 1]  # (128, r, s, jp, h)
        G = G.transpose(0, 4, 3, 1, 2)  # (128, h, jp, 2, 2)
        Pc = G[:, :, 3] @ G[:, :, 2] @ G[:, :, 1] @ G[:, :, 0]  # (128, 2, 2, 2)
        Pk = Pc.transpose(1, 0, 2, 3).reshape(NPAIR, 2, 2)  # k = 128*h + p
        M = Pk @ M
    ei = np.exp(1j * phases_in.astype(np.float64)).reshape(NPAIR, 2)
    eo = np.exp(1j * phases_out.astype(np.float64)).reshape(NPAIR, 2)
    G = (eo[:, :, None] * M * ei[:, None, :]).astype(np.complex64)
    out = np.zeros((N, N), np.complex64)
    idx = np.arange(NPAIR) * 2
    out[idx, idx] = G[:, 0, 0]
    out[idx, idx + 1] = G[:, 0, 1]
    out[idx + 1, idx] = G[:, 1, 0]
    out[idx + 1, idx + 1] = G[:, 1, 1]
    return out


# ---------------------------------------------------------------------------
# bass module
# ---------------------------------------------------------------------------

_NC = None


def _tree_level(nc, pool, f16, S, lanes):
    """One combine level: Snext[j] = S[odd] @ S[even] (2x2 complex, fp16).

    S: tile [128, 8*lanes], free = (comp, half, t): comp = 4r+2s+part,
    half: even/odd step (bit-reversed order), t: packed (pos, h) lanes.
    Returns Snext [128, 8*lanes//2] with free = (comp, t).

    Every operand view collapses to <= 3 free dims after stride merging
    (DVE TENSOR3D limit), with packed stride-1 last dims (fp16 2x mode).
    """
    T = lanes // 2  # packed lanes per half
    P = pool.tile([128, 4 * 8 * T], f16)  # (pa, pb, r, m, s, t)
    Q = pool.tile([128, 2 * 8 * T], f16)  # (m, qp, r, s, t)
    Sn = pool.tile([128, 8 * T], f16)

    scv = S[:].rearrange("p (c half t) -> p c half t", c=8, half=2)
    # A = odd steps (left factor), comp = 4r+2m+pa ; B = even, comp = 4m+2s+pb
    A = scv[:, :, 1].rearrange("p (r m pa) t -> p r m pa t", r=2, m=2, pa=2)
    B = scv[:, :, 0].rearrange("p (m s pb) t -> p m s pb t", m=2, s=2, pb=2)
    pv = P[:].rearrange(
        "p (pa pb r m s t) -> p pa pb r m s t", pa=2, pb=2, r=2, m=2, s=2
    )
    for pa in range(2):
        for pb in range(2):
            # nest (r, m, s, t): op1 merges (r,m), op2 merges (m,s)
            op1 = A[:, :, :, pa].unsqueeze(3).broadcast_to((128, 2, 2, 2, T))
            op2 = B[:, :, :, pb].unsqueeze(1).broadcast_to((128, 2, 2, 2, T))
            nc.vector.tensor_mul(pv[:, pa, pb], op1, op2)
    qv = Q[:].rearrange("p (m qp r s t) -> p m qp r s t", m=2, qp=2, r=2, s=2)
    # nest (m, r, s, t): P slices merge (s,t), Q slices merge (r,s,t)
    pt = pv.transpose([0, 1, 2, 4, 3, 5, 6])  # (p, pa, pb, m, r, s, t)
    nc.vector.tensor_sub(qv[:, :, 0], pt[:, 0, 0], pt[:, 1, 1])
    nc.vector.tensor_add(qv[:, :, 1], pt[:, 0, 1], pt[:, 1, 0])
    # Snext comp = 4r+2s+qp, summed over m; nest (r, s, qp, t)
    snv = Sn[:].rearrange("p (r s qp t) -> p r s qp t", r=2, s=2, qp=2)
    qt = qv.transpose([0, 1, 3, 4, 2, 5])  # (p, m, r, s, qp, t)
    nc.vector.tensor_add(snv, qt[:, 0], qt[:, 1])
    return Sn


def _build_module():
    import concourse.bass as bass
    import concourse.bacc as bacc
    import concourse.mybir as mybir
    from concourse import tile

    f32 = mybir.dt.float32
    f16 = mybir.dt.float16
    AF = mybir.ActivationFunctionType
    ALU = mybir.AluOpType

    nc = bacc.Bacc("TRN2", target_bir_lowering=False, debug=False, num_devices=NCORE)
    inp_ext = nc.dram_tensor("inp", [128, 384], f32, kind="ExternalInput").ap()
    out_ext = nc.dram_tensor("out", [128, 64], f16, kind="ExternalOutput").ap()

    C10 = float(np.log(10.0) / 20.0)  # a = exp(-C10 * l)
    R = float(1.0 / np.sqrt(2.0))

    with tile.TileContext(nc) as tc:
        with tc.tile_pool(name="sbuf", bufs=1) as pool:
            bnegpi = pool.tile([128, 1], f32)
            bhalfpi = pool.tile([128, 1], f32)
            nc.gpsimd.memset(bnegpi[:], -float(np.pi))
            nc.gpsimd.memset(bhalfpi[:], float(np.pi / 2))

            inp = pool.tile([128, 384], f32)
            # m-block first on SP (it heads the DVE chain), l second on SP,
            # alpha/beta on the Pool SWDGE queue
            nc.sync.dma_start(inp[:, 128:256], inp_ext[:, 128:256])
            nc.sync.dma_start(inp[:, 0:128], inp_ext[:, 0:128])
            nc.gpsimd.dma_start(inp[:, 256:384], inp_ext[:, 256:384])

            # ---- construction ----
            mm = inp[:, 128:256]  # [m0|m1]
            m2 = pool.tile([128, 128], f32)
            nc.vector.tensor_mul(m2[:], mm, mm)
            ev = pool.tile([128, 128], f32)  # even part r*(1 - m^2/8)
            nc.vector.tensor_scalar(ev[:], m2[:], -R / 8.0, R, ALU.mult, ALU.add)
            t1p = pool.tile([128, 128], f32)  # r/2 + r*m^2/16
            nc.vector.tensor_scalar(t1p[:], m2[:], R / 16.0, R / 2.0, ALU.mult, ALU.add)
            ov = pool.tile([128, 128], f32)  # odd part m*(r/2 + r*m^2/16)
            nc.vector.tensor_mul(ov[:], mm, t1p[:])
            q4 = pool.tile([128, 256], f32)  # [q0p|q1p|q0m|q1m]
            nc.vector.tensor_add(q4[:, 0:128], ev[:], ov[:])
            nc.vector.tensor_sub(q4[:, 128:256], ev[:], ov[:])

            # phase prep (ACT needs |ph - pi| for the cos recipe); early so the
            # Sin activations run while the DVE does the amplitude cubic
            psh = pool.tile([128, 128], f32)
            nc.vector.tensor_scalar_add(psh[:], inp[:, 256:384], -float(np.pi))
            abs2 = pool.tile([128, 128], f32)
            nc.vector.scalar_tensor_tensor(abs2[:], psh[:], -1.0, psh[:],
                                           ALU.mult, ALU.max)
            trig = pool.tile([128, 256], f32)  # [ca'|cb'|sa'|sb'] (primed)
            nc.scalar.activation(trig[:, 128:256], inp[:, 256:384], AF.Sin, bias=bnegpi[:])
            nc.scalar.activation(trig[:, 0:128], abs2[:], AF.Sin, bias=bhalfpi[:], scale=-1.0)

            # amplitude exp as cubic: e = (1 + x) + l^2*(C^2/2 - C^3 l/6),
            # x = -C*l, |x| <= 0.058 (err < 5e-7)
            ll = inp[:, 0:128]  # [l0|l1]
            pA = pool.tile([128, 128], f32)
            nc.vector.tensor_scalar(pA[:], ll, -(C10**3) / 6.0, C10**2 / 2.0,
                                    ALU.mult, ALU.add)
            pX = pool.tile([128, 128], f32)
            nc.vector.tensor_scalar(pX[:], ll, -C10, 1.0, ALU.mult, ALU.add)
            pL2 = pool.tile([128, 128], f32)
            nc.vector.tensor_mul(pL2[:], ll, ll)
            pP = pool.tile([128, 128], f32)
            nc.vector.tensor_mul(pP[:], pL2[:], pA[:])
            expa = pool.tile([128, 128], f32)  # [a0|a1]
            nc.vector.tensor_add(expa[:], pX[:], pP[:])

            tk = pool.tile([128, 256], f32)  # [t0|t1|k0|k1]
            tkv = tk[:].rearrange("p (x m u) -> p x m u", x=2, m=2)
            eop = expa[:].rearrange("p (m u) -> p m u", m=2).unsqueeze(1).broadcast_to((128, 2, 2, 64))
            q4v = q4[:].rearrange("p (x m u) -> p x m u", x=2, m=2)
            nc.vector.tensor_mul(tkv, eop, q4v)
            # coef [tt|tk|kt|kk]: (c1,c0) -> tk1[c1] * tk0[c0]
            coef = pool.tile([128, 256], f32)
            cfv = coef[:].rearrange("p (c1 c0 u) -> p c1 c0 u", c1=2, c0=2)
            op1 = tk[:].rearrange("p (x m u) -> p x m u", x=2, m=2)[:, :, 1, :].unsqueeze(2).broadcast_to((128, 2, 2, 64))
            op2 = tk[:].rearrange("p (x m u) -> p x m u", x=2, m=2)[:, :, 0, :].unsqueeze(1).broadcast_to((128, 2, 2, 64))
            nc.vector.tensor_mul(cfv, op1, op2)

            # PR[cq, taut] = coef[cq] * trig[taut]; kk row (cq=3) on Pool
            pr = pool.tile([128, 768], f32)
            prkk = pool.tile([128, 256], f32)
            prv = pr[:].rearrange("p (cq t u) -> p cq t u", cq=3, t=4)
            cop = coef[:, 0:192].rearrange("p (cq u) -> p cq u", cq=3).unsqueeze(2).broadcast_to((128, 3, 4, 64))
            top = trig[:].rearrange("p (t u) -> p t u", t=4).unsqueeze(1).broadcast_to((128, 3, 4, 64))
            nc.vector.tensor_mul(prv, cop, top)
            pkv = prkk[:].rearrange("p (t u) -> p t u", t=4)
            kop = coef[:, 192:256].unsqueeze(1).broadcast_to((128, 4, 64))
            top2 = trig[:].rearrange("p (t u) -> p t u", t=4)
            nc.gpsimd.tensor_mul(pkv, kop, top2)

            # ---- combine into S' (fp16, comp = 4r+2s+part) ----
            S = pool.tile([128, 512], f16)
            sv = S[:].rearrange("p (c u) -> p c u", c=8)
            # re {S00re:0, S11re:6}: PR[tt, ca/cb] - PR[kk, cb/ca]
            nc.vector.tensor_sub(sv[:, 0:8:6], prv[:, 0, 0:2], pkv[:, 0:2][:, ::-1])
            # im {S00im:1, S11im:7}: PR[tt, sa/sb] - PR[kk, sb/sa]
            nc.vector.tensor_sub(sv[:, 1:8:6], prv[:, 0, 2:4], pkv[:, 2:4][:, ::-1])
            # {S01im:3, S10im:5}: PR[tk/kt, ca] + PR[kt/tk, cb]
            nc.vector.tensor_add(sv[:, 3:6:2], prv[:, 1:3, 0], prv[:, 1:3, 1][:, ::-1])
            # {S01re:2, S10re:4}: -(PR[tk/kt, sa] + PR[kt/tk, sb])
            nc.vector.scalar_tensor_tensor(sv[:, 2:5:2], prv[:, 1:3, 2], -1.0,
                                           prv[:, 1:3, 3][:, ::-1],
                                           ALU.mult, ALU.subtract)

            # ---- fp16 combine tree: 32 steps -> 4 partials ----
            S2 = _tree_level(nc, pool, f16, S, 64)
            S3 = _tree_level(nc, pool, f16, S2, 32)
            O = _tree_level(nc, pool, f16, S3, 16)
            nc.sync.dma_start(out_ext[:], O[:])

    nc.finalize()
    return nc


def _get_module():
    global _NC
    if _NC is None:
        _NC = _build_module()
    return _NC


def kernel(ht_in_phase, ht_out_phase, ht_full_phases, mmi_i_losses, mmi_imbalances):
    from concourse.bass_utils import run_bass_kernel_spmd

    nc = _get_module()
    losses = np.asarray(mmi_i_losses, np.float32)
    imbal = np.asarray(mmi_imbalances, np.float32)
    phases = np.asarray(ht_full_phases, np.float32)
    in_maps = [{"inp": _host_prep(c, losses, imbal, phases)} for c in range(NCORE)]
    res = run_bass_kernel_spmd(nc, in_maps, list(range(NCORE)))
    Os = [res.results[c]["out"] for c in range(NCORE)]
    return _host_finish(
        Os, np.asarray(ht_in_phase, np.float32), np.asarray(ht_out_phase, np.float32)
    )
